# revision 1
# baseline (speedup 1.0000x reference)
"""Inverse 2D Haar DWT (idwt2) Trainium2 Bass kernel.

Full inputs: approximation/detail_h/detail_v/detail_d each [8, 64, 128, 128] f32.
Full output: [8, 64, 256, 256] f32 with out 2x2 blocks:
  x00 = (a + v + h + d)/2   at [2i,   2j]
  x01 = (a - v + h - d)/2   at [2i,   2j+1]
  x10 = (a + v - h - d)/2   at [2i+1, 2j]
  x11 = (a - v - h + d)/2   at [2i+1, 2j+1]

Sharding: batch dim across 8 cores (1 batch each), no communication.

Per-core layout trick: view the (64,128,128) input as [128, 8192] where
partition P = 2*c + (i>=64) holds rows i in [64*(P%2), 64*(P%2)+64) of
channel c = P//2, each partition's data fully contiguous in DRAM. The
(64,256,256) output viewed as [128, 32768] has the *same* partition map
(P = 2*c + (i2>=128)), so input loads and output stores are both fully
contiguous DMAs with multi-KB descriptors.

Butterfly: with p=(a+h)/2, r=(a-h)/2, q=(v+d)/2, s=(v-d)/2:
  x00=p+q, x01=p-q, x10=r+s, x11=r-s.
ACT prescales as=a/2, vs=v/2; DVE does 4 scalar_tensor_tensor +
4 tensor_tensor ops, writing the final 4 directly into the interleaved
[row-pair packed] output tile so the store DMA is linear.
"""

import numpy as np

B, C, H, W = 8, 64, 128, 128
N_CORES = 8
R = 16  # rows (of 64 per partition block) processed per group
G = 64 // R

_cache = {}


def _build():
    import concourse.bacc as bacc
    import concourse.tile as tile
    from concourse import mybir

    fp32 = mybir.dt.float32
    add = mybir.AluOpType.add
    sub = mybir.AluOpType.subtract
    mult = mybir.AluOpType.mult

    nc = bacc.Bacc("TRN2", target_bir_lowering=False, debug=False)

    names = ["approximation", "detail_h", "detail_v", "detail_d"]
    ins = {
        n: nc.dram_tensor(n, [128, 64 * 128], fp32, kind="ExternalInput").ap()
        for n in names
    }
    out = nc.dram_tensor("out", [128, 128 * 256], fp32, kind="ExternalOutput").ap()

    FD = R * 128  # free-dim elems per input tile

    with tile.TileContext(nc) as tc:
        with (
            tc.tile_pool(name="inp", bufs=2) as inp,
            tc.tile_pool(name="tmp", bufs=2) as tmp,
            tc.tile_pool(name="outp", bufs=2) as outp,
        ):
            for g in range(G):
                isl = slice(g * FD, (g + 1) * FD)
                ta = inp.tile([128, FD], fp32, tag="a")
                th = inp.tile([128, FD], fp32, tag="h")
                tv = inp.tile([128, FD], fp32, tag="v")
                td = inp.tile([128, FD], fp32, tag="d")
                nc.sync.dma_start(out=ta[:], in_=ins["approximation"][:, isl])
                nc.sync.dma_start(out=th[:], in_=ins["detail_h"][:, isl])
                nc.sync.dma_start(out=tv[:], in_=ins["detail_v"][:, isl])
                nc.sync.dma_start(out=td[:], in_=ins["detail_d"][:, isl])

                tas = tmp.tile([128, FD], fp32, tag="as")
                tvs = tmp.tile([128, FD], fp32, tag="vs")
                nc.scalar.mul(tas[:], ta[:], 0.5)  # as = a/2
                nc.scalar.mul(tvs[:], tv[:], 0.5)  # vs = v/2

                # p -> ta, r -> th, q -> tv, s -> td (reuse input tiles)
                nc.vector.scalar_tensor_tensor(ta[:], th[:], 0.5, tas[:], mult, add)
                nc.vector.scalar_tensor_tensor(th[:], th[:], -0.5, tas[:], mult, add)
                nc.vector.scalar_tensor_tensor(tv[:], td[:], 0.5, tvs[:], mult, add)
                nc.vector.scalar_tensor_tensor(td[:], td[:], -0.5, tvs[:], mult, add)

                to = outp.tile([128, R * 512], fp32, tag="o")
                o3 = to[:].rearrange("p (r w) -> p r w", w=512)
                p3 = ta[:].rearrange("p (r w) -> p r w", w=128)
                r3 = th[:].rearrange("p (r w) -> p r w", w=128)
                q3 = tv[:].rearrange("p (r w) -> p r w", w=128)
                s3 = td[:].rearrange("p (r w) -> p r w", w=128)

                nc.vector.tensor_tensor(o3[:, :, 0:256:2], p3, q3, add)  # x00
                nc.vector.tensor_tensor(o3[:, :, 1:256:2], p3, q3, sub)  # x01
                nc.vector.tensor_tensor(o3[:, :, 256:512:2], r3, s3, add)  # x10
                nc.vector.tensor_tensor(o3[:, :, 257:512:2], r3, s3, sub)  # x11

                nc.sync.dma_start(
                    out=out[:, g * R * 512 : (g + 1) * R * 512], in_=to[:]
                )

    nc.compile()
    return nc


def kernel(approximation, detail_h, detail_v, detail_d):
    from concourse.bass_utils import run_bass_kernel_spmd

    if "nc" not in _cache:
        _cache["nc"] = _build()
    nc = _cache["nc"]

    full = {
        "approximation": approximation,
        "detail_h": detail_h,
        "detail_v": detail_v,
        "detail_d": detail_d,
    }
    in_maps = [
        {
            k: np.ascontiguousarray(v[b]).reshape(128, 64 * 128)
            for k, v in full.items()
        }
        for b in range(N_CORES)
    ]
    res = run_bass_kernel_spmd(nc, in_maps, list(range(N_CORES)))
    out = np.stack(
        [res.results[b]["out"].reshape(C, 2 * H, 2 * W) for b in range(N_CORES)]
    )
    return out.astype(np.float32, copy=False)


# revision 4
# speedup vs baseline: 1.2432x; 1.2432x over previous
"""Inverse 2D Haar DWT (idwt2) Trainium2 Bass kernel.

Full inputs: approximation/detail_h/detail_v/detail_d each [8, 64, 128, 128] f32.
Full output: [8, 64, 256, 256] f32 with out 2x2 blocks:
  x00 = (a + v + h + d)/2   at [2i,   2j]
  x01 = (a - v + h - d)/2   at [2i,   2j+1]
  x10 = (a + v - h - d)/2   at [2i+1, 2j]
  x11 = (a - v - h + d)/2   at [2i+1, 2j+1]

Sharding: batch dim across 8 cores (1 batch each), no communication.

Per-core layout trick: view the (64,128,128) input as [128, 8192] where
partition P = 2*c + (i>=64) holds rows i in [64*(P%2), 64*(P%2)+64) of
channel c = P//2, each partition's data fully contiguous in DRAM. The
(64,256,256) output viewed as [128, 32768] has the *same* partition map
(P = 2*c + (i2>=128)), so input loads and output stores are both fully
contiguous DMAs with multi-KB descriptors.

Butterfly: with p=(a+h)/2, r=(a-h)/2, q=(v+d)/2, s=(v-d)/2:
  x00=p+q, x01=p-q, x10=r+s, x11=r-s.
ACT prescales as=a/2, vs=v/2; DVE does 4 scalar_tensor_tensor +
4 tensor_tensor ops, writing the final 4 directly into the interleaved
[row-pair packed] output tile so the store DMA is linear.
"""

import numpy as np

B, C, H, W = 8, 64, 128, 128
N_CORES = 8
R = 8  # rows (of 64 per partition block) processed per group
G = 64 // R

_cache = {}


def _build():
    import concourse.bacc as bacc
    import concourse.tile as tile
    from concourse import mybir

    fp32 = mybir.dt.float32
    add = mybir.AluOpType.add
    sub = mybir.AluOpType.subtract
    mult = mybir.AluOpType.mult

    nc = bacc.Bacc("TRN2", target_bir_lowering=False, debug=False)

    names = ["approximation", "detail_h", "detail_v", "detail_d"]
    ins = {
        n: nc.dram_tensor(n, [128, 64 * 128], fp32, kind="ExternalInput").ap()
        for n in names
    }
    out = nc.dram_tensor("out", [128, 128 * 256], fp32, kind="ExternalOutput").ap()

    FD = R * 128  # free-dim elems per input tile

    with tile.TileContext(nc) as tc:
        with (
            tc.tile_pool(name="inp", bufs=4) as inp,
            tc.tile_pool(name="tmp", bufs=2) as tmp,
            tc.tile_pool(name="outp", bufs=3) as outp,
        ):
            for g in range(G):
                isl = slice(g * FD, (g + 1) * FD)
                ta = inp.tile([128, FD], fp32, tag="a")
                th = inp.tile([128, FD], fp32, tag="h")
                tv = inp.tile([128, FD], fp32, tag="v")
                td = inp.tile([128, FD], fp32, tag="d")
                nc.sync.dma_start(out=ta[:], in_=ins["approximation"][:, isl])
                nc.sync.dma_start(out=th[:], in_=ins["detail_h"][:, isl])
                nc.sync.dma_start(out=tv[:], in_=ins["detail_v"][:, isl])
                nc.sync.dma_start(out=td[:], in_=ins["detail_d"][:, isl])

                tas = tmp.tile([128, FD], fp32, tag="as")
                tvs = tmp.tile([128, FD], fp32, tag="vs")
                nc.scalar.mul(tas[:], ta[:], 0.5)  # as = a/2
                nc.scalar.mul(tvs[:], tv[:], 0.5)  # vs = v/2

                # p -> ta, r -> th, q -> tv, s -> td (reuse input tiles)
                nc.vector.scalar_tensor_tensor(ta[:], th[:], 0.5, tas[:], mult, add)
                nc.vector.scalar_tensor_tensor(th[:], th[:], -0.5, tas[:], mult, add)
                nc.vector.scalar_tensor_tensor(tv[:], td[:], 0.5, tvs[:], mult, add)
                nc.vector.scalar_tensor_tensor(td[:], td[:], -0.5, tvs[:], mult, add)

                to = outp.tile([128, R * 512], fp32, tag="o")
                o3 = to[:].rearrange("p (r w) -> p r w", w=512)
                p3 = ta[:].rearrange("p (r w) -> p r w", w=128)
                r3 = th[:].rearrange("p (r w) -> p r w", w=128)
                q3 = tv[:].rearrange("p (r w) -> p r w", w=128)
                s3 = td[:].rearrange("p (r w) -> p r w", w=128)

                nc.vector.tensor_tensor(o3[:, :, 0:256:2], p3, q3, add)  # x00
                nc.vector.tensor_tensor(o3[:, :, 1:256:2], p3, q3, sub)  # x01
                nc.vector.tensor_tensor(o3[:, :, 256:512:2], r3, s3, add)  # x10
                nc.vector.tensor_tensor(o3[:, :, 257:512:2], r3, s3, sub)  # x11

                # separate HWDGE ring (qActDynamicHW) so stores don't
                # head-of-line block the input loads on the sync ring
                nc.scalar.dma_start(
                    out=out[:, g * R * 512 : (g + 1) * R * 512], in_=to[:]
                )

    nc.compile()
    return nc


def kernel(approximation, detail_h, detail_v, detail_d):
    from concourse.bass_utils import run_bass_kernel_spmd

    if "nc" not in _cache:
        _cache["nc"] = _build()
    nc = _cache["nc"]

    full = {
        "approximation": approximation,
        "detail_h": detail_h,
        "detail_v": detail_v,
        "detail_d": detail_d,
    }
    in_maps = [
        {
            k: np.ascontiguousarray(v[b]).reshape(128, 64 * 128)
            for k, v in full.items()
        }
        for b in range(N_CORES)
    ]
    res = run_bass_kernel_spmd(nc, in_maps, list(range(N_CORES)))
    out = np.stack(
        [res.results[b]["out"].reshape(C, 2 * H, 2 * W) for b in range(N_CORES)]
    )
    return out.astype(np.float32, copy=False)
